# revision 7
# baseline (speedup 1.0000x reference)
# Multi-head attention (B=2, S=2048, D=1024, H=16) on 8 Trainium2 NeuronCores.
#
# v3: fp16 datapath + software-pipelined emission order + global tile
# pools (cross-rep pipelining) + fine-grained filler schedule + PE warm-up.
#
# Sharding: tensor-parallel over heads (2 heads/core), chunked AllGather of
# normalized per-head context, feature-sharded output projection (128 output
# channels/core). Same math as v1; the changes are scheduling + dtype:
#
#  - All DMA'd tensors (x, weights, collective buffers, output) and all
#    SBUF-resident matmul operands are float16: halves HBM traffic and SBUF
#    footprint; PE streaming speed is unchanged (1 col/cycle for fp16/f32r)
#    but LDWEIGHTS gets FWL (2x) and moving operands may span 1024 cols.
#  - Emission order: K(batch0) -> Q(block0) -> attention blocks, with the
#    remaining projections (Q b0, KVQ batch1) emitted as "filler" units
#    between attention score/ctx groups. The scalar engine (exp: 16.8M
#    elements/core ~ 133us) is the long pole; fillers keep PE busy under it
#    and attention starts at ~10us instead of ~67us.
#  - PSUM budget (8 banks): scores tag 2 bufs x [128,1024] = 4 banks,
#    ctx tag 2 bufs x [65,512] = 2 banks, misc (proj/transpose/out-proj)
#    tag 2 bufs x [128,512] = 2 banks.
#
# Softmax: scores are O(1) -> exp without max-subtraction (exact up to fp
# rounding). Masking: V rows pre-multiplied by mask; mask appended as 65th
# lhsT column so the denominator accumulates in the same PE pass as ctx.

import numpy as np

B, S, D, H, HD = 2, 2048, 1024, 16, 64
N_CORES = 8
BS = B * S            # 4096 total positions
DPC = D // N_CORES    # 128 channels per core (2 heads)
QB = 512              # query-block columns
NBLK = BS // QB       # 8 query blocks (4 per batch)
NKT = S // 128        # 16 key tiles per batch element
NE = D // 128         # 8 contraction chunks of the model dim
NG = 2                # key tiles per exp group

_CACHE = {}


def _build_nc(nreps=1, no_collective=False, cc_blocks=1):
    import concourse.mybir as mybir
    import concourse.tile as tile
    from concourse import bacc
    from concourse.masks import make_identity

    F32 = mybir.dt.float32
    F16 = mybir.dt.float16
    F8 = mybir.dt.float8e4
    DR = mybir.MatmulPerfMode.DoubleRow
    EXP = mybir.ActivationFunctionType.Exp

    nc = bacc.Bacc(None, target_bir_lowering=False, num_devices=N_CORES)

    # x, host-pre-swizzled per 512-position chunk: [sc][p, c, q]
    xT_d = nc.dram_tensor("xT", [BS // QB, 128, NE, QB], F16, kind="ExternalInput")
    # mask, host-pre-transposed: [128, 32] partition-major per 128-pos tile
    mask_d = nc.dram_tensor("maskf", [128, BS // 128], F32, kind="ExternalInput")
    w_d = {}
    for nm in ("wq", "wk", "wv", "wo"):
        # host-pre-swizzled to the SBUF lhsT layout: [128, 8*128]
        w_d[nm] = nc.dram_tensor(nm, [128, D], F16, kind="ExternalInput")
    # biases combined: [128, 4] columns = (q, k, v, o)
    b_d = nc.dram_tensor("ball", [DPC, 4], F32, kind="ExternalInput")
    out_d = nc.dram_tensor("outT", [DPC, BS], F16, kind="ExternalOutput")

    # Collective bounce buffers, one per cc_blocks query blocks (and per rep).
    cc_in = [
        nc.dram_tensor(f"cc_in{j}", [DPC, cc_blocks * QB], F16)
        for j in range(NBLK * nreps // cc_blocks)
    ]
    cc_out = [
        nc.dram_tensor(f"cc_out{j}", [D, cc_blocks * QB], F16, addr_space="Shared")
        for j in range(NBLK * nreps // cc_blocks)
    ]
    rgroup = [list(range(N_CORES))]

    def emit_rep(tc, pers, pools, pending, xtiles, rep):
        w_sb, b_sb, maskt, ident, qr8, kr8, q8, k8, vT, vp = pers
        xp, ptp, smal, cgp, ps_s, ps_c, ps_m = pools
        if True:

            def load_x(sc, xrep=rep):
                if (xrep, sc) in xtiles:
                    return xtiles[(xrep, sc)]
                xct = xp.tile([128, NE, QB], F16, name=f"xct{xrep}_{sc}", tag="xc")
                nc.sync.dma_start(xct[:], xT_d[sc])
                xtiles[(xrep, sc)] = xct
                return xct

            def load_w(nm):
                nc.sync.dma_start(w_sb[nm][:], w_d[nm][:])

            def proj(nm, sc, lo=0, hi=QB, xrep=rep):
                # one projection (q/k/v) of a [lo:hi] column slice of one
                # 512-position chunk; V goes to the persistent vT staging
                # buffer, transposes are separate. Q/K are cast to fp8 in a
                # row-layout staging buffer, then reshape-DMA'd into the
                # DoubleRow split layout [32(h-dim), sc, ksub, pos].
                xct = load_x(sc, xrep)
                cols = slice(QB * sc + lo, QB * sc + hi)
                ps = ps_m.tile([128, QB], F32, name=f"ps_{nm}{sc}", tag="m")
                for e in range(NE):
                    ech = slice(128 * e, 128 * (e + 1))
                    nc.tensor.matmul(
                        ps[:, lo:hi], w_sb["w" + nm][:, ech], xct[:, e, lo:hi],
                        start=(e == 0), stop=(e == NE - 1),
                    )
                if nm == "v":
                    nc.vector.tensor_scalar_add(vT[:, cols], ps[:, lo:hi], b_sb[:, 2:3])
                    return
                r8, s8, bi = (qr8, q8, 0) if nm == "q" else (kr8, k8, 1)
                nc.vector.tensor_scalar_add(r8[:, cols], ps[:, lo:hi], b_sb[:, bi : bi + 1])
                for h in range(2):
                    for ks in range(2):
                        nc.sync.dma_start(
                            s8[32 * h : 32 * h + 32, sc, ks, lo:hi],
                            r8[64 * h + 32 * ks : 64 * h + 32 * ks + 32, cols],
                        )

            def vtr(sc):
                # transpose + mask-fold the 4 key tiles of one V chunk
                for u in range(4):
                    t = 4 * sc + u
                    vtp = ps_m.tile([128, 128], F16, name=f"vtp{t}", tag="m")
                    nc.tensor.transpose(
                        vtp[:], vT[:, 128 * t : 128 * (t + 1)], ident[:]
                    )
                    for h in range(2):
                        nc.vector.tensor_scalar_mul(
                            vp[h][:, 65 * t : 65 * t + 64],
                            vtp[:, 64 * h : 64 * (h + 1)],
                            maskt[:, t : t + 1],
                        )
                        nc.vector.tensor_copy(
                            vp[h][:, 65 * t + 64 : 65 * t + 65],
                            maskt[:, t : t + 1],
                        )

            # ---- lead-in: K for batch 0, Q for block 0, V for batch 0 ----
            # (weight DMAs for q/v/o are emitted mid-stream, behind the
            # x-chunk loads the K projections need first)
            if rep == 0:
                load_w("wk")
                load_x(0)
                nc.sync.dma_start(b_sb[:], b_d[:])
                load_x(1)
                load_w("wq")
                load_x(2)
                load_x(3)
                load_w("wv")
                nc.sync.dma_start(maskt[:], mask_d[:])
                load_w("wo")
                # PE warm-up during the initial DMA wait: dummy transposes
                # keep the HAM activity window busy so the first real matmuls
                # run at full clock (results are never read)
                for wi in range(6):
                    wps = ps_m.tile([128, 128], F16, name=f"warm{wi}", tag="m")
                    nc.tensor.transpose(wps[:], ident[:], ident[:])
            # first scores group only needs kT cols 0..255 (key tiles 0-1);
            # the second half of k0 follows as the first filler. Reps > 0
            # get their lead-in from the PREVIOUS rep's prelude fillers.
            if rep == 0:
                proj("k", 0, 0, QB // 2)
                proj("q", 0)

            # Filler units scheduled per attention group (popped between the
            # group's exp and ctx emissions). Units are small (proj = 8 mms,
            # vtr = 4 transposes) so the PE burst per group stays near the
            # ACT period. Constraints: scores of group gg need K chunk gg//2
            # one group early; ctx of group gg needs vtr chunk gg//2 at the
            # group; batch-1 operands complete well before block j4 (gctr 33).
            P, VT = proj, vtr
            k0b = lambda *_: proj("k", 0, QB // 2, QB)
            sched = {
                1: [(k0b, None, None), (P, "v", 0), (VT, None, 0)],
                2: [(P, "k", 1), (P, "v", 1)],
                3: [(VT, None, 1), (P, "k", 2)],
                4: [(P, "v", 2), (P, "q", 1)],
                5: [(VT, None, 2), (P, "k", 3)],
                6: [(P, "v", 3)],
                7: [(VT, None, 3)],
                9: [(P, "k", 4)], 11: [(P, "q", 2)], 13: [(P, "k", 5)],
                15: [(P, "q", 3)], 17: [(P, "k", 6)], 19: [(P, "k", 7)],
                # batch-1 K/Q must precede their consumer blocks; V/vtr units
                # land just-in-time inside block j4 (its ctx group gg needs
                # vtr chunk 4+gg//2 at that group; j5..j7 re-read them later,
                # so everything must exist by gctr 40)
                21: [(P, "q", 4)], 23: [(P, "v", 4)], 25: [(VT, None, 4)],
                27: [(P, "q", 5)], 29: [(P, "v", 5)], 31: [(VT, None, 5)],
                33: [(P, "q", 6)], 34: [(P, "v", 6)], 35: [(VT, None, 6)],
                36: [(P, "q", 7)], 37: [(P, "v", 7), (VT, None, 7)],
            }

            op_sched = {7: -1, 20: 0, 28: 1, 38: 2, 42: 3, 50: 4, 56: 5, 62: 6}

            if rep + 1 < nreps:
                sched[45] = [(lambda *_: load_x(0, rep + 1), None, None)]
                sched[47] = [
                    (lambda *_: proj("k", 0, 0, QB // 2, xrep=rep + 1),
                     None, None)
                ]
                sched[49] = [
                    (lambda *_: proj("q", 0, xrep=rep + 1), None, None)
                ]
                sched[51] = [
                    (lambda *_: proj("k", 0, QB // 2, QB, xrep=rep + 1),
                     None, None)
                ]
                sched[53] = [(lambda *_: load_x(1, rep + 1), None, None)]
            if rep > 0:
                # k0b was precomputed by the previous rep's prelude
                sched[1] = [e for e in sched[1] if e[0] is not k0b]

            def pop_fillers(g):
                if g in op_sched and pending:
                    out_proj(*pending.pop(0))
                for fn, nm, sc in sched.pop(g, ()):
                    if fn is vtr:
                        vtr(sc)
                    elif fn is proj:
                        fn(nm, sc)
                    else:
                        fn()


            def out_proj(j, jj):
                # Output projection for one query block (feature-sharded).
                pj, half = divmod(jj, cc_blocks)
                hcols = slice(QB * half, QB * (half + 1))
                qcols = slice(QB * j, QB * (j + 1))
                cg = cgp.tile([128, NE, QB], F16, name=f"cg{j}", tag="cg")
                nc.sync.dma_start(
                    cg[:],
                    cc_out[pj][:, hcols].rearrange("(c p) q -> p c q", p=128),
                )
                o_ps = ps_m.tile([128, QB], F32, name=f"o_ps{j}", tag="m")
                for c in range(NE):
                    nc.tensor.matmul(
                        o_ps[:], w_sb["wo"][:, 128 * c : 128 * (c + 1)],
                        cg[:, c, :], start=(c == 0), stop=(c == NE - 1),
                    )
                oc = smal.tile([128, QB], F16, name=f"oc{j}", tag="oc")
                nc.vector.tensor_scalar_add(oc[:], o_ps[:], b_sb[:, 3:4])
                nc.sync.dma_start(out_d[:, qcols], oc[:])

            gctr = 0
            for j in range(NBLK):
                b = j // (NBLK // B)
                jj = rep * NBLK + j
                qcols = slice(QB * j, QB * (j + 1))
                ctx_ps = [
                    ps_c.tile([65, QB], F32, name=f"ctx{j}_{h}", tag="c")
                    for h in range(2)
                ]

                def emit_ctx(g, pts):
                    for h in range(2):
                        for u in range(NG):
                            kt = NG * g + u
                            vtile = NKT * b + kt
                            nc.tensor.matmul(
                                ctx_ps[h][:],
                                vp[h][:, 65 * vtile : 65 * (vtile + 1)],
                                pts[h][:, QB * u : QB * (u + 1)],
                                start=(g == 0 and u == 0),
                                stop=(g == NKT // NG - 1 and u == NG - 1),
                            )

                # software-pipelined by one group: ctx of group g-1 is
                # emitted after the scores of group g, so PE only ever waits
                # on the PREVIOUS group's exp (via the scores psum WAR), not
                # the current one
                prev = None
                for g in range(NKT // NG):
                    s_ps = [None, None]
                    for h in range(2):
                        sp = ps_s.tile(
                            [128, NG * QB], F32, name=f"s{j}_{g}_{h}", tag="s"
                        )
                        hrow = slice(32 * h, 32 * h + 32)
                        for u in range(NG):
                            kt = NG * g + u
                            kpos = S * b + 128 * kt
                            ksc, koff = divmod(kpos, QB)
                            nc.tensor.matmul(
                                sp[:, QB * u : QB * (u + 1)],
                                k8[hrow, ksc, :, koff : koff + 128],
                                q8[hrow, j, :, :],
                                start=True,
                                stop=True,
                                perf_mode=DR,
                                tile_position=(32 * h, 0),
                            )
                        s_ps[h] = sp
                    pts = [None, None]
                    for h in range(2):
                        pt = ptp.tile([128, NG * QB], F16, name=f"pt{h}", tag="pt")
                        nc.scalar.activation(pt[:], s_ps[h][:], EXP, scale=0.125)
                        pts[h] = pt
                    gctr += 1
                    # pop scheduled fillers between the exp and ctx emissions
                    # (a ctx group may consume vp tiles a filler produces)
                    pop_fillers(gctr)
                    if prev is not None:
                        emit_ctx(*prev)
                    prev = (g, pts)
                emit_ctx(*prev)


                # ---- normalize ----
                cn = smal.tile([128, QB], F16, name=f"cn{j}", tag="cn")
                for h in range(2):
                    den = smal.tile([1, QB], F32, name="den", tag="den")
                    # stage the denominator row via a regular-op copy first
                    # (custom-DVE ops drop the AP base_partition; PSUM
                    # partition offsets must be 32-aligned)
                    nc.vector.tensor_copy(den[:], ctx_ps[h][64:65, :])
                    recip = smal.tile([1, QB], F32, name="recip", tag="recip")
                    nc.vector.reciprocal_approx_fast(recip[:], den[:])
                    rb = smal.tile([64, QB], F32, name="rb", tag="rb")
                    nc.gpsimd.partition_broadcast(rb[:], recip[:])
                    nc.vector.tensor_mul(
                        cn[64 * h : 64 * (h + 1), :], ctx_ps[h][0:64, :], rb[:]
                    )
                pj, half = divmod(jj, cc_blocks)
                hcols = slice(QB * half, QB * (half + 1))
                nc.sync.dma_start(cc_in[pj][:, hcols], cn[:])
                if half == cc_blocks - 1:
                    if no_collective:
                        # local loopback stand-in (wrong data on cores > 0)
                        for c in range(NE):
                            nc.sync.dma_start(
                                cc_out[pj][128 * c : 128 * (c + 1), :],
                                cc_in[pj][:],
                            )
                    else:
                        nc.gpsimd.collective_compute(
                            "AllGather",
                            mybir.AluOpType.bypass,
                            replica_groups=rgroup,
                            ins=[cc_in[pj][:].opt()],
                            outs=[cc_out[pj][:].opt()],
                        )

                pending.append((j, jj))

            return out_proj

    with tile.TileContext(nc) as tc:
        with tc.tile_pool(name="persist", bufs=1) as pp:
            # Weight shards: [128, 1024] tiles, contraction chunk e at
            # columns 128e..128e+128 (lhsT chunk = w[:, 128e:128e+128]).
            # DMAs for weights/biases/mask are emitted inside emit_rep (rep 0)
            # in critical-path order; only the tiles are allocated here.
            w_sb = {
                nm: pp.tile([128, D], F16, name=f"{nm}_sb")
                for nm in ("wk", "wq", "wv", "wo")
            }
            b_sb = pp.tile([DPC, 4], F32, name="ball_sb")
            maskt = pp.tile([128, BS // 128], F32, name="maskt")
            ident = pp.tile([128, 128], F16, name="ident")
            make_identity(nc, ident[:])

            # fp8 Q/K: row-layout staging + DoubleRow split layout
            # [32(h-dim part), sc, ksub, pos] per head pair (partitions
            # 0-31 = head 0, 32-63 = head 1).
            qr8 = pp.tile([128, BS], F8, name="qr8")
            kr8 = pp.tile([128, BS], F8, name="kr8")
            q8 = pp.tile([64, NBLK, 2, QB], F8, name="q8")
            k8 = pp.tile([64, NBLK, 2, QB], F8, name="k8")
            vT = pp.tile([128, BS], F16, name="vT")
            # V' per head: [128, 65] per key tile; col 64 is the mask column.
            vp = [
                pp.tile([128, (BS // 128) * 65], F16, name=f"vp{h}")
                for h in range(2)
            ]
            pers = (w_sb, b_sb, maskt, ident, qr8, kr8, q8, k8, vT, vp)
            with (
                tc.tile_pool(name="xcol", bufs=8) as xp,
                tc.tile_pool(name="ptp", bufs=4) as ptp,
                tc.tile_pool(name="smal", bufs=4) as smal,
                tc.tile_pool(name="ctxg", bufs=3) as cgp,
                tc.tile_pool(name="ps_s", bufs=2, space="PSUM") as ps_s,
                tc.tile_pool(name="ps_c", bufs=2, space="PSUM") as ps_c,
                tc.tile_pool(name="ps_m", bufs=2, space="PSUM") as ps_m,
            ):
                pools = (xp, ptp, smal, cgp, ps_s, ps_c, ps_m)
                pending = []
                xtiles = {}
                for rep in range(nreps):
                    drain = emit_rep(tc, pers, pools, pending, xtiles, rep)
                while pending:
                    drain(*pending.pop(0))


    nc.compile()
    return nc


def _get_nc(nreps=1, no_collective=False, cc_blocks=1):
    key = (nreps, no_collective, cc_blocks)
    if key not in _CACHE:
        _CACHE[key] = _build_nc(nreps, no_collective, cc_blocks)
    return _CACHE[key]


def _make_in_maps(x, mask, Wq, bq, Wk, bk, Wv, bv, Wo, bo):
    f32, f16 = np.float32, np.float16
    x = np.asarray(x, f32)
    xT = x.reshape(BS, D).T.astype(f16)  # [D, BS]
    # pre-swizzle x into the SBUF chunk layout: [sc, p, c(=e), q]
    xs = np.ascontiguousarray(
        xT.reshape(NE, 128, BS // QB, QB).transpose(2, 1, 0, 3)
    )
    maskf = np.ascontiguousarray(
        np.asarray(mask).astype(f32).reshape(BS // 128, 128).T
    )
    Ws = {"wq": np.asarray(Wq, f32), "wk": np.asarray(Wk, f32), "wv": np.asarray(Wv, f32), "wo": np.asarray(Wo, f32)}
    ball = np.stack(
        [np.asarray(b, f32) for b in (bq, bk, bv, bo)], axis=1
    )  # [D, 4]
    in_maps = []
    for r in range(N_CORES):
        rows = slice(DPC * r, DPC * (r + 1))
        m = {"xT": xs, "maskf": maskf, "ball": np.ascontiguousarray(ball[rows])}
        for nm, W in Ws.items():
            # SBUF lhsT layout pre-swizzle: [p, c*128+d], chunk c = rows
            # 128c..128c+128 of W[rows].T
            wt = W[rows].T.astype(f16)  # [D, DPC]
            m[nm] = np.ascontiguousarray(
                wt.reshape(NE, 128, DPC).transpose(1, 0, 2).reshape(128, D)
            )
        in_maps.append(m)
    return in_maps


def kernel(x, mask, Wq, bq, Wk, bk, Wv, bv, Wo, bo):
    from concourse import bass_utils

    nc = _get_nc()
    in_maps = _make_in_maps(x, mask, Wq, bq, Wk, bk, Wv, bv, Wo, bo)
    try:
        res = bass_utils.run_bass_kernel_spmd(
            nc, in_maps, core_ids=list(range(N_CORES))
        )
    except Exception:
        # one retry: a previously-crashed run can leave a core wedged and
        # fail the first execution afterwards
        res = bass_utils.run_bass_kernel_spmd(
            nc, in_maps, core_ids=list(range(N_CORES))
        )
    outT = np.concatenate([res.results[r]["outT"] for r in range(N_CORES)], axis=0)
    return np.ascontiguousarray(outT.astype(np.float32).T).reshape(B, S, D)



# revision 26
# speedup vs baseline: 1.2092x; 1.2092x over previous
# Multi-head attention (B=2, S=2048, D=1024, H=16) on 8 Trainium2 NeuronCores.
#
# v3: fp16 datapath + software-pipelined emission order + global tile
# pools (cross-rep pipelining) + fine-grained filler schedule + PE warm-up.
#
# Sharding: tensor-parallel over heads (2 heads/core), chunked AllGather of
# normalized per-head context, feature-sharded output projection (128 output
# channels/core). Same math as v1; the changes are scheduling + dtype:
#
#  - All DMA'd tensors (x, weights, collective buffers, output) and all
#    SBUF-resident matmul operands are float16: halves HBM traffic and SBUF
#    footprint; PE streaming speed is unchanged (1 col/cycle for fp16/f32r)
#    but LDWEIGHTS gets FWL (2x) and moving operands may span 1024 cols.
#  - Emission order: K(batch0) -> Q(block0) -> attention blocks, with the
#    remaining projections (Q b0, KVQ batch1) emitted as "filler" units
#    between attention score/ctx groups. The scalar engine (exp: 16.8M
#    elements/core ~ 133us) is the long pole; fillers keep PE busy under it
#    and attention starts at ~10us instead of ~67us.
#  - PSUM budget (8 banks): scores tag 2 bufs x [128,1024] = 4 banks,
#    ctx tag 2 bufs x [65,512] = 2 banks, misc (proj/transpose/out-proj)
#    tag 2 bufs x [128,512] = 2 banks.
#
# Softmax: scores are O(1) -> exp without max-subtraction (exact up to fp
# rounding). Masking: V rows pre-multiplied by mask; mask appended as 65th
# lhsT column so the denominator accumulates in the same PE pass as ctx.

import numpy as np

B, S, D, H, HD = 2, 2048, 1024, 16, 64
N_CORES = 8
BS = B * S            # 4096 total positions
DPC = D // N_CORES    # 128 channels per core (2 heads)
QB = 512              # query-block columns
NBLK = BS // QB       # 8 query blocks (4 per batch)
NKT = S // 128        # 16 key tiles per batch element
NE = D // 128         # 8 contraction chunks of the model dim
NG = 2                # key tiles per exp group

_CACHE = {}

# exp(x/8) ~= (1 + c1 x + c2 x^2 + c3 x^3)^4 on raw scores x in [-18, 18]
# (max rel err 2.3e-3 on [-16, 16]); computed as a custom DVE op so part
# of the softmax exp runs on the Vector engine instead of the (saturated)
# scalar/ACT engine. Coefficients from a relative-error weighted LS fit.
_EXP_C1 = 3.12988957e-02
_EXP_C2 = 4.97213707e-04
_EXP_C3 = 4.88579931e-06
_EXP_OP = None


def _register_exp_op():
    global _EXP_OP
    if _EXP_OP is not None:
        return _EXP_OP
    import concourse.dve_ops as dve_ops

    for op in dve_ops.OPS:
        if op.name == "ANT_EXP_P34":
            _EXP_OP = op
            return op
    from concourse.dve_spec import Spec, Src0, C0, C1, C2, One, sq
    from concourse.dve_spec import lower, _has_src1
    from concourse.dve_uop import DveOpSpec

    body = sq(sq(((C0 * Src0 + C1) * Src0 + C2) * Src0 + One))

    def ref(in0, in1, c0, c1, c2):
        p = ((c0 * in0 + c1) * in0 + c2) * in0 + 1.0
        return (p * p) * (p * p)

    spec = Spec(body=body, reference=ref)
    row = 1 + len(dve_ops.OPS)
    shas = {}
    for ver in ("v3", "v4"):
        s = DveOpSpec(
            name="ANT_EXP_P34", opcode=row, uops=lower(spec, ver=ver),
            rd1_en=_has_src1(spec),
        )
        shas[ver] = s.sha(ver)
    op = dve_ops.DveOp("ANT_EXP_P34", spec, subdim=False, uops_sha=shas)
    dve_ops.OPS.append(op)
    dve_ops.CUSTOM_DVE_SPECS[op.name] = op.spec
    dve_ops._SUB_OPCODE_FOR_NAME[op.name] = row
    _EXP_OP = op
    return op


def _build_nc(nreps=1, no_collective=False, cc_blocks=1, fp8=True,
              skip_collective=False, act_frac=1.0, dve_exp_mod=4):
    # skip_collective: remove collectives entirely (timing diagnostic;
    #   out_proj reads unwritten DRAM, results wrong on all cores).
    # act_frac: fraction of exp groups actually computed (timing
    #   diagnostic; <1.0 gives wrong results).
    # dve_exp_mod: every dve_exp_mod-th exp unit runs on the Vector engine
    #   via the polynomial approximation (0 = all exp on ACT).
    import concourse.mybir as mybir
    import concourse.tile as tile
    from concourse import bacc
    from concourse.masks import make_identity

    F32 = mybir.dt.float32
    F16 = mybir.dt.float16
    F8 = mybir.dt.float8e4
    DR = mybir.MatmulPerfMode.DoubleRow
    EXP = mybir.ActivationFunctionType.Exp

    nc = bacc.Bacc(None, target_bir_lowering=False, num_devices=N_CORES)

    # x, host-pre-swizzled per 512-position chunk: [sc][p, c, q]
    xT_d = nc.dram_tensor("xT", [BS // QB, 128, NE, QB], F16, kind="ExternalInput")
    # mask, host-pre-transposed: [128, 32] partition-major per 128-pos tile
    mask_d = nc.dram_tensor("maskf", [128, BS // 128], F32, kind="ExternalInput")
    w_d = {}
    for nm in ("wq", "wk", "wv", "wo"):
        # host-pre-swizzled to the SBUF lhsT layout: [128, 8*128]
        w_d[nm] = nc.dram_tensor(nm, [128, D], F16, kind="ExternalInput")
    # biases combined: [128, 4] columns = (q, k, v, o)
    b_d = nc.dram_tensor("ball", [DPC, 4], F32, kind="ExternalInput")
    out_d = nc.dram_tensor("outT", [DPC, BS], F16, kind="ExternalOutput")

    # Collective bounce buffers, one per cc_blocks query blocks (and per rep).
    cc_in = [
        nc.dram_tensor(f"cc_in{j}", [DPC, cc_blocks * QB], F16)
        for j in range(NBLK * nreps // cc_blocks)
    ]
    cc_out = [
        nc.dram_tensor(f"cc_out{j}", [D, cc_blocks * QB], F16, addr_space="Shared")
        for j in range(NBLK * nreps // cc_blocks)
    ]
    rgroup = [list(range(N_CORES))]

    def emit_rep(tc, pers, pools, pending, xtiles, rep):
        w_sb, b_sb, maskt, ident, qr8, kr8, q8, k8, qT, kT, vT, vp = pers
        xp, ptp, smal, cgp, ps_s, ps_c, ps_m = pools
        if True:

            def load_x(sc, xrep=rep):
                if (xrep, sc) in xtiles:
                    return xtiles[(xrep, sc)]
                xct = xp.tile([128, NE, QB], F16, name=f"xct{xrep}_{sc}", tag="xc")
                nc.sync.dma_start(xct[:], xT_d[sc])
                xtiles[(xrep, sc)] = xct
                return xct

            def load_w(nm):
                nc.sync.dma_start(w_sb[nm][:], w_d[nm][:])

            def proj(nm, sc, lo=0, hi=QB, xrep=rep):
                # one projection (q/k/v) of a [lo:hi] column slice of one
                # 512-position chunk; V goes to the persistent vT staging
                # buffer, transposes are separate. Q/K are cast to fp8 in a
                # row-layout staging buffer, then reshape-DMA'd into the
                # DoubleRow split layout [32(h-dim), sc, ksub, pos].
                xct = load_x(sc, xrep)
                cols = slice(QB * sc + lo, QB * sc + hi)
                ps = ps_m.tile([128, QB], F32, name=f"ps_{nm}{sc}", tag="m")
                for e in range(NE):
                    ech = slice(128 * e, 128 * (e + 1))
                    nc.tensor.matmul(
                        ps[:, lo:hi], w_sb["w" + nm][:, ech], xct[:, e, lo:hi],
                        start=(e == 0), stop=(e == NE - 1),
                    )
                if nm == "v":
                    nc.vector.tensor_scalar_add(vT[:, cols], ps[:, lo:hi], b_sb[:, 2:3])
                    return
                bi = 0 if nm == "q" else 1
                if not fp8:
                    dst = qT if nm == "q" else kT
                    nc.vector.tensor_scalar_add(dst[:, cols], ps[:, lo:hi], b_sb[:, bi : bi + 1])
                    return
                # psum rows for q/k are host-permuted to r' = 64h + 2p + k
                # (k-minor), so one DMA per head de-interleaves into the
                # DoubleRow split layout [32(p), ksub, pos].
                r8, s8 = (qr8, q8) if nm == "q" else (kr8, k8)
                nc.vector.tensor_scalar_add(r8[:, cols], ps[:, lo:hi], b_sb[:, bi : bi + 1])
                for h in range(2):
                    nc.sync.dma_start(
                        s8[32 * h : 32 * h + 32, sc, :, lo:hi],
                        r8[64 * h : 64 * h + 64, cols],
                    )

            def vtr(sc):
                # transpose + mask-fold the 4 key tiles of one V chunk
                for u in range(4):
                    t = 4 * sc + u
                    vtp = ps_m.tile([128, 128], F16, name=f"vtp{t}", tag="m")
                    nc.tensor.transpose(
                        vtp[:], vT[:, 128 * t : 128 * (t + 1)], ident[:]
                    )
                    for h in range(2):
                        nc.vector.tensor_scalar_mul(
                            vp[h][:, 65 * t : 65 * t + 64],
                            vtp[:, 64 * h : 64 * (h + 1)],
                            maskt[:, t : t + 1],
                        )
                        nc.vector.tensor_copy(
                            vp[h][:, 65 * t + 64 : 65 * t + 65],
                            maskt[:, t : t + 1],
                        )

            # ---- lead-in: K for batch 0, Q for block 0, V for batch 0 ----
            # (weight DMAs for q/v/o are emitted mid-stream, behind the
            # x-chunk loads the K projections need first)
            if rep == 0:
                load_w("wk")
                load_x(0)
                nc.sync.dma_start(b_sb[:], b_d[:])
                load_x(1)
                load_w("wq")
                load_x(2)
                load_x(3)
                load_w("wv")
                nc.sync.dma_start(maskt[:], mask_d[:])
                load_w("wo")
                # PE warm-up during the initial DMA wait: dummy transposes
                # keep the HAM activity window busy so the first real matmuls
                # run at full clock (results are never read)
                for wi in range(6):
                    wps = ps_m.tile([128, 128], F16, name=f"warm{wi}", tag="m")
                    nc.tensor.transpose(wps[:], ident[:], ident[:])
            # first scores group only needs kT cols 0..255 (key tiles 0-1);
            # the second half of k0 follows as the first filler. Reps > 0
            # get their lead-in from the PREVIOUS rep's prelude fillers.
            if rep == 0:
                proj("k", 0, 0, QB // 2)
                proj("q", 0)

            # Filler units scheduled per attention group (popped between the
            # group's exp and ctx emissions). Units are small (proj = 8 mms,
            # vtr = 4 transposes) so the PE burst per group stays near the
            # ACT period. Constraints: scores of group gg need K chunk gg//2
            # one group early; ctx of group gg needs vtr chunk gg//2 at the
            # group; batch-1 operands complete well before block j4 (gctr 33).
            P, VT = proj, vtr
            k0b = lambda *_: proj("k", 0, QB // 2, QB)
            sched = {
                1: [(k0b, None, None), (P, "v", 0), (VT, None, 0)],
                2: [(P, "k", 1), (P, "v", 1)],
                3: [(VT, None, 1), (P, "k", 2)],
                4: [(P, "v", 2), (P, "q", 1)],
                5: [(VT, None, 2), (P, "k", 3)],
                6: [(P, "v", 3)],
                7: [(VT, None, 3)],
                9: [(P, "k", 4)], 11: [(P, "q", 2)], 13: [(P, "k", 5)],
                15: [(P, "q", 3)], 17: [(P, "k", 6)], 19: [(P, "k", 7)],
                # batch-1 K/Q must precede their consumer blocks; V/vtr units
                # land just-in-time inside block j4 (its ctx group gg needs
                # vtr chunk 4+gg//2 at that group; j5..j7 re-read them later,
                # so everything must exist by gctr 40)
                21: [(P, "q", 4)], 23: [(P, "v", 4)], 25: [(VT, None, 4)],
                27: [(P, "q", 5)], 29: [(P, "v", 5)], 31: [(VT, None, 5)],
                33: [(P, "q", 6)], 34: [(P, "v", 6)], 35: [(VT, None, 6)],
                36: [(P, "q", 7)], 37: [(P, "v", 7), (VT, None, 7)],
            }

            op_sched = {7: -1, 20: 0, 28: 1, 38: 2, 42: 3, 50: 4, 56: 5, 62: 6}

            if rep + 1 < nreps:
                sched[45] = [(lambda *_: load_x(0, rep + 1), None, None)]
                sched[47] = [
                    (lambda *_: proj("k", 0, 0, QB // 2, xrep=rep + 1),
                     None, None)
                ]
                sched[49] = [
                    (lambda *_: proj("q", 0, xrep=rep + 1), None, None)
                ]
                sched[51] = [
                    (lambda *_: proj("k", 0, QB // 2, QB, xrep=rep + 1),
                     None, None)
                ]
                sched[53] = [(lambda *_: load_x(1, rep + 1), None, None)]
            if rep > 0:
                # k0b was precomputed by the previous rep's prelude
                sched[1] = [e for e in sched[1] if e[0] is not k0b]

            def pop_fillers(g):
                if g in op_sched and pending:
                    out_proj(*pending.pop(0))
                for fn, nm, sc in sched.pop(g, ()):
                    if fn is vtr:
                        vtr(sc)
                    elif fn is proj:
                        fn(nm, sc)
                    else:
                        fn()


            def out_proj(j, jj):
                # Output projection for one query block (feature-sharded).
                pj, half = divmod(jj, cc_blocks)
                hcols = slice(QB * half, QB * (half + 1))
                qcols = slice(QB * j, QB * (j + 1))
                cg = cgp.tile([128, NE, QB], F16, name=f"cg{j}", tag="cg")
                nc.sync.dma_start(
                    cg[:],
                    cc_out[pj][:, hcols].rearrange("(c p) q -> p c q", p=128),
                )
                o_ps = ps_m.tile([128, QB], F32, name=f"o_ps{j}", tag="m")
                for c in range(NE):
                    nc.tensor.matmul(
                        o_ps[:], w_sb["wo"][:, 128 * c : 128 * (c + 1)],
                        cg[:, c, :], start=(c == 0), stop=(c == NE - 1),
                    )
                oc = smal.tile([128, QB], F16, name=f"oc{j}", tag="oc")
                nc.vector.tensor_scalar_add(oc[:], o_ps[:], b_sb[:, 3:4])
                nc.sync.dma_start(out_d[:, qcols], oc[:])

            gctr = 0
            for j in range(NBLK):
                b = j // (NBLK // B)
                jj = rep * NBLK + j
                qcols = slice(QB * j, QB * (j + 1))
                ctx_ps = [
                    ps_c.tile([65, QB], F32, name=f"ctx{j}_{h}", tag="c")
                    for h in range(2)
                ]

                def emit_ctx(g, pts):
                    for h in range(2):
                        for u in range(NG):
                            kt = NG * g + u
                            vtile = NKT * b + kt
                            nc.tensor.matmul(
                                ctx_ps[h][:],
                                vp[h][:, 65 * vtile : 65 * (vtile + 1)],
                                pts[h][:, QB * u : QB * (u + 1)],
                                start=(g == 0 and u == 0),
                                stop=(g == NKT // NG - 1 and u == NG - 1),
                            )

                # software-pipelined by one group: ctx of group g-1 is
                # emitted after the scores of group g, so PE only ever waits
                # on the PREVIOUS group's exp (via the scores psum WAR), not
                # the current one
                prev = None
                last_pts = None
                for g in range(NKT // NG):
                    s_ps = [None, None]
                    for h in range(2):
                        sp = ps_s.tile(
                            [128, NG * QB], F32, name=f"s{j}_{g}_{h}", tag="s"
                        )
                        for u in range(NG):
                            kt = NG * g + u
                            kpos = S * b + 128 * kt
                            if fp8:
                                hrow = slice(32 * h, 32 * h + 32)
                                ksc, koff = divmod(kpos, QB)
                                nc.tensor.matmul(
                                    sp[:, QB * u : QB * (u + 1)],
                                    k8[hrow, ksc, :, koff : koff + 128],
                                    q8[hrow, j, :, :],
                                    start=True,
                                    stop=True,
                                    perf_mode=DR,
                                    tile_position=(32 * h, 0),
                                )
                            else:
                                hrow = slice(64 * h, 64 * (h + 1))
                                nc.tensor.matmul(
                                    sp[:, QB * u : QB * (u + 1)],
                                    kT[hrow, kpos : kpos + 128],
                                    qT[hrow, qcols],
                                    start=True,
                                    stop=True,
                                    tile_position=(64 * h, 0),
                                )
                        s_ps[h] = sp
                    pts = [None, None]
                    do_act = (g * act_frac) % 1.0 < act_frac if act_frac < 1.0 else True
                    for h in range(2):
                        if do_act or last_pts is None:
                            pt = ptp.tile([128, NG * QB], F16, name=f"pt{h}", tag="pt")
                            state["exp_ctr"] += 1
                            if dve_exp_mod and state["exp_ctr"] % dve_exp_mod == 1:
                                nc.vector._custom_dve(
                                    exp_op, out=pt[:], in0=s_ps[h][:],
                                    s0=_EXP_C3, s1=_EXP_C2, imm2=_EXP_C1,
                                )
                            else:
                                nc.scalar.activation(pt[:], s_ps[h][:], EXP, scale=0.125)
                        else:
                            pt = last_pts[h]
                        pts[h] = pt
                    last_pts = pts
                    gctr += 1
                    # pop scheduled fillers between the exp and ctx emissions
                    # (a ctx group may consume vp tiles a filler produces)
                    pop_fillers(gctr)
                    if prev is not None:
                        emit_ctx(*prev)
                    prev = (g, pts)
                emit_ctx(*prev)


                # ---- normalize ----
                cn = smal.tile([128, QB], F16, name=f"cn{j}", tag="cn")
                for h in range(2):
                    den = smal.tile([1, QB], F32, name="den", tag="den")
                    # stage the denominator row via a regular-op copy first
                    # (custom-DVE ops drop the AP base_partition; PSUM
                    # partition offsets must be 32-aligned)
                    nc.vector.tensor_copy(den[:], ctx_ps[h][64:65, :])
                    recip = smal.tile([1, QB], F32, name="recip", tag="recip")
                    nc.vector.reciprocal_approx_fast(recip[:], den[:])
                    rb = smal.tile([64, QB], F32, name="rb", tag="rb")
                    nc.gpsimd.partition_broadcast(rb[:], recip[:])
                    nc.vector.tensor_mul(
                        cn[64 * h : 64 * (h + 1), :], ctx_ps[h][0:64, :], rb[:]
                    )
                pj, half = divmod(jj, cc_blocks)
                hcols = slice(QB * half, QB * (half + 1))
                nc.sync.dma_start(cc_in[pj][:, hcols], cn[:])
                if half == cc_blocks - 1:
                    if skip_collective:
                        pass
                    elif no_collective:
                        # local loopback stand-in (wrong data on cores > 0)
                        for c in range(NE):
                            nc.sync.dma_start(
                                cc_out[pj][128 * c : 128 * (c + 1), :],
                                cc_in[pj][:],
                            )
                    else:
                        nc.gpsimd.collective_compute(
                            "AllGather",
                            mybir.AluOpType.bypass,
                            replica_groups=rgroup,
                            ins=[cc_in[pj][:].opt()],
                            outs=[cc_out[pj][:].opt()],
                        )

                pending.append((j, jj))

            return out_proj

    with tile.TileContext(nc) as tc:
        with tc.tile_pool(name="persist", bufs=1) as pp:
            # Weight shards: [128, 1024] tiles, contraction chunk e at
            # columns 128e..128e+128 (lhsT chunk = w[:, 128e:128e+128]).
            # DMAs for weights/biases/mask are emitted inside emit_rep (rep 0)
            # in critical-path order; only the tiles are allocated here.
            w_sb = {
                nm: pp.tile([128, D], F16, name=f"{nm}_sb")
                for nm in ("wk", "wq", "wv", "wo")
            }
            b_sb = pp.tile([DPC, 4], F32, name="ball_sb")
            maskt = pp.tile([128, BS // 128], F32, name="maskt")
            ident = pp.tile([128, 128], F16, name="ident")
            make_identity(nc, ident[:])

            if fp8:
                # fp8 Q/K: row-layout staging + DoubleRow split layout
                # [32(h-dim part), sc, ksub, pos] per head pair (partitions
                # 0-31 = head 0, 32-63 = head 1).
                qr8 = pp.tile([128, BS], F8, name="qr8")
                kr8 = pp.tile([128, BS], F8, name="kr8")
                q8 = pp.tile([64, NBLK, 2, QB], F8, name="q8")
                k8 = pp.tile([64, NBLK, 2, QB], F8, name="k8")
                qT = kT = None
            else:
                qT = pp.tile([128, BS], F16, name="qT")
                kT = pp.tile([128, BS], F16, name="kT")
                qr8 = kr8 = q8 = k8 = None
            vT = pp.tile([128, BS], F16, name="vT")
            # V' per head: [128, 65] per key tile; col 64 is the mask column.
            vp = [
                pp.tile([128, (BS // 128) * 65], F16, name=f"vp{h}")
                for h in range(2)
            ]
            pers = (w_sb, b_sb, maskt, ident, qr8, kr8, q8, k8, qT, kT, vT, vp)
            with (
                tc.tile_pool(name="xcol", bufs=8) as xp,
                tc.tile_pool(name="ptp", bufs=4) as ptp,
                tc.tile_pool(name="smal", bufs=4) as smal,
                tc.tile_pool(name="ctxg", bufs=3) as cgp,
                tc.tile_pool(name="ps_s", bufs=2, space="PSUM") as ps_s,
                tc.tile_pool(name="ps_c", bufs=2, space="PSUM") as ps_c,
                tc.tile_pool(name="ps_m", bufs=2, space="PSUM") as ps_m,
            ):
                pools = (xp, ptp, smal, cgp, ps_s, ps_c, ps_m)
                pending = []
                xtiles = {}
                state = {"exp_ctr": 0}
                exp_op = _register_exp_op() if dve_exp_mod else None
                for rep in range(nreps):
                    drain = emit_rep(tc, pers, pools, pending, xtiles, rep)
                while pending:
                    drain(*pending.pop(0))


    nc.compile()
    return nc


def _get_nc(nreps=1, no_collective=False, cc_blocks=1, fp8=True,
            skip_collective=False, act_frac=1.0, dve_exp_mod=4):
    key = (nreps, no_collective, cc_blocks, fp8, skip_collective, act_frac,
           dve_exp_mod)
    if key not in _CACHE:
        _CACHE[key] = _build_nc(nreps, no_collective, cc_blocks, fp8,
                                skip_collective, act_frac, dve_exp_mod)
    return _CACHE[key]


def _make_in_maps(x, mask, Wq, bq, Wk, bk, Wv, bv, Wo, bo):
    f32, f16 = np.float32, np.float16
    x = np.asarray(x, f32)
    xT = x.reshape(BS, D).T.astype(f16)  # [D, BS]
    # pre-swizzle x into the SBUF chunk layout: [sc, p, c(=e), q]
    xs = np.ascontiguousarray(
        xT.reshape(NE, 128, BS // QB, QB).transpose(2, 1, 0, 3)
    )
    maskf = np.ascontiguousarray(
        np.asarray(mask).astype(f32).reshape(BS // 128, 128).T
    )
    Ws = {"wq": np.asarray(Wq, f32), "wk": np.asarray(Wk, f32), "wv": np.asarray(Wv, f32), "wo": np.asarray(Wo, f32)}
    # q/k channel permutation: psum row r' = 64h + 2p + k holds original
    # local channel 64h + 32k + p (k-minor interleave for the fp8 reshape)
    rp = np.arange(DPC)
    qk_perm = 64 * (rp // 64) + 32 * (rp % 2) + (rp % 64) // 2
    ball = np.stack(
        [np.asarray(b, f32) for b in (bq, bk, bv, bo)], axis=1
    )  # [D, 4]
    in_maps = []
    for r in range(N_CORES):
        rows = slice(DPC * r, DPC * (r + 1))
        br = np.ascontiguousarray(ball[rows])
        br[:, 0] = br[qk_perm, 0]
        br[:, 1] = br[qk_perm, 1]
        m = {"xT": xs, "maskf": maskf, "ball": br}
        for nm, W in Ws.items():
            # SBUF lhsT layout pre-swizzle: [p, c*128+d], chunk c = rows
            # 128c..128c+128 of W[rows].T
            wt = W[rows].T.astype(f16)  # [D, DPC]
            if nm in ("wq", "wk"):
                wt = wt[:, qk_perm]
            m[nm] = np.ascontiguousarray(
                wt.reshape(NE, 128, DPC).transpose(1, 0, 2).reshape(128, D)
            )
        in_maps.append(m)
    return in_maps


def kernel(x, mask, Wq, bq, Wk, bk, Wv, bv, Wo, bo):
    from concourse import bass_utils

    nc = _get_nc()
    in_maps = _make_in_maps(x, mask, Wq, bq, Wk, bk, Wv, bv, Wo, bo)
    try:
        res = bass_utils.run_bass_kernel_spmd(
            nc, in_maps, core_ids=list(range(N_CORES))
        )
    except Exception:
        # one retry: a previously-crashed run can leave a core wedged and
        # fail the first execution afterwards
        res = bass_utils.run_bass_kernel_spmd(
            nc, in_maps, core_ids=list(range(N_CORES))
        )
    outT = np.concatenate([res.results[r]["outT"] for r in range(N_CORES)], axis=0)
    return np.ascontiguousarray(outT.astype(np.float32).T).reshape(B, S, D)

